# revision 34
# baseline (speedup 1.0000x reference)
"""Fused multi-head cross-attention with relation branch, sharded over 8 NeuronCores.

Sharding: data-parallel over batch (4) x tensor-parallel over head halves (2).
Core c handles batch c//2, heads [8*(c%2), 8*(c%2)+8). Each core computes its
partial output projection; the host sums the two partials per batch and adds bo.

Device data flow (per core):
  - q/k/rk projections emitted transposed: qT/kT/rkT [512 local dims, 1024 L]
    (4 chunks of 128 dims = head pairs (2dc, 2dc+1) at partitions 0-63/64-127)
  - v/rv projections emitted natural: [1024 LK, 512 dims], stored per lk-chunk
    with a ones column appended per head ([v_h | 1] of width 65) so the PV
    matmul's row 64 accumulates the softmax denominator for free.
  - scores computed transposed sT[lk, lq] = kT.T @ qT per head, two heads
    row-packed on the PE array (K=64 each at array rows 0-63 / 64-127).
  - exp + mask + 1/sqrt(dk) fused into one ACT op per score tile:
    p = exp(s*scale + bias[lk]) with bias = 0 / -1e9 from the key mask.
  - x_att^T accumulated in PSUM over lk chunks: [v_h|1].T @ p -> [65, lq].
  - softmax denominators batch-reciprocated on 128 DVE lanes via an SBUF->SBUF
    DMA reshape, broadcast over 64 partitions via gpsimd DMAs, then the
    two branches are combined with DVE fma ops.
  - output projection yT = WoT.T @ x_final accumulated over 4 dim chunks.
  - ~8 warmup matmuls on a memset tile right after the preamble keep the PE
    HAM clock gate open while the first input DMAs are still in flight, and
    the input DMA queue leads with xq0/wq0 (split across the sync and scalar
    queues) so real matmuls start as early as possible.
"""

import math

import numpy as np

B, LQ, LK, D, H = 4, 1024, 1024, 1024, 16
DK = D // H
SCALE = 1.0 / math.sqrt(DK)
N_CORES = 8
HD = D // 2  # local dims per core (8 heads * 64)
# Keys are compacted host-side: only unmasked keys are shipped (padded to LKP
# with dummy rows whose mask bias is -1e9, so exp()=0 -> exact same math).
LKP = 640
NM = LKP // 128  # lk chunks

_CACHE = {}


def _build_program(lkp=LKP):
    import concourse.bacc as bacc
    import concourse.mybir as mybir
    import concourse.tile as tile

    LKP = lkp
    NM = LKP // 128

    f32 = mybir.dt.float32
    bf16 = mybir.dt.bfloat16
    Exp = mybir.ActivationFunctionType.Exp
    Add = mybir.AluOpType.add
    Mult = mybir.AluOpType.mult

    nc = bacc.Bacc(
        "TRN2",
        target_bir_lowering=False,
        debug=False,
        enable_asserts=False,
        num_devices=N_CORES,
    )

    # DRAM I/O (per-core shapes; host shards/pre-transposes/casts).
    xqT = nc.dram_tensor("xqT", [D, LQ], bf16, kind="ExternalInput").ap()
    xkT = nc.dram_tensor("xkT", [D, LKP], bf16, kind="ExternalInput").ap()
    xrT = nc.dram_tensor("xrT", [D, LKP], bf16, kind="ExternalInput").ap()
    xvT = nc.dram_tensor("xvT", [D, LKP], bf16, kind="ExternalInput").ap()
    wqT = nc.dram_tensor("wqT", [D, HD], bf16, kind="ExternalInput").ap()
    wkT = nc.dram_tensor("wkT", [D, HD], bf16, kind="ExternalInput").ap()
    wrkT = nc.dram_tensor("wrkT", [D, HD], bf16, kind="ExternalInput").ap()
    wvT = nc.dram_tensor("wvT", [D, HD], bf16, kind="ExternalInput").ap()
    wrvT = nc.dram_tensor("wrvT", [D, HD], bf16, kind="ExternalInput").ap()
    woT = nc.dram_tensor("woT", [HD, D], bf16, kind="ExternalInput").ap()
    bq_pc = nc.dram_tensor("bq_pc", [128, 4], f32, kind="ExternalInput").ap()
    bk_pc = nc.dram_tensor("bk_pc", [128, 4], f32, kind="ExternalInput").ap()
    brk_pc = nc.dram_tensor("brk_pc", [128, 4], f32, kind="ExternalInput").ap()
    bv_bc = nc.dram_tensor("bv_bc", [128, HD], f32, kind="ExternalInput").ap()
    brv_bc = nc.dram_tensor("brv_bc", [128, HD], f32, kind="ExternalInput").ap()
    maskb = nc.dram_tensor("maskb", [128, NM], f32, kind="ExternalInput").ap()
    yT = nc.dram_tensor("yT", [D, LQ], f32, kind="ExternalOutput").ap()
    scr1 = nc.dram_tensor("scr1", [8, 2048], f32, kind="Internal").ap()
    scr2 = nc.dram_tensor("scr2", [8, 2048], f32, kind="Internal").ap()

    with tile.TileContext(nc) as tc:
        from contextlib import ExitStack

        with ExitStack() as ctx:
            # Persistent SBUF tensors.
            persist = ctx.enter_context(tc.tile_pool(name="persist", bufs=1))
            qT_sb = persist.tile([128, 4 * LQ], bf16, tag="qT")
            kT_sb = persist.tile([128, 4 * LKP], bf16, tag="kT")
            rkT_sb = persist.tile([128, 4 * LKP], bf16, tag="rkT")
            v_sb = persist.tile([128, NM * 8 * 65], bf16, tag="v")
            rv_sb = persist.tile([128, NM * 8 * 65], bf16, tag="rv")
            xf_sb = persist.tile([128, 4 * LQ], bf16, tag="xf")
            maskb_sb = persist.tile([128, NM], f32, tag="maskb")
            bq_sb = persist.tile([128, 4], f32, tag="bq")
            bk_sb = persist.tile([128, 4], f32, tag="bk")
            brk_sb = persist.tile([128, 4], f32, tag="brk")
            bv_sb = persist.tile([128, HD], f32, tag="bv")
            brv_sb = persist.tile([128, HD], f32, tag="brv")
            warm_sb = persist.tile([128, 512], bf16, tag="warm")

            # Memset first: warmup matmuls depend only on this.
            nc.vector.memset(warm_sb[:], 0.125)

            # Small parameter DMAs on the gpsimd queue (off the critical
            # sync queue that carries the first input chunks).
            nc.gpsimd.dma_start(out=maskb_sb[:], in_=maskb)
            nc.gpsimd.dma_start(out=bq_sb[:], in_=bq_pc)
            nc.gpsimd.dma_start(out=bk_sb[:], in_=bk_pc)
            nc.gpsimd.dma_start(out=brk_sb[:], in_=brk_pc)
            nc.gpsimd.dma_start(out=bv_sb[:], in_=bv_bc)
            nc.gpsimd.dma_start(out=brv_sb[:], in_=brv_bc)

            v4 = v_sb[:].rearrange("p (m h c) -> p m h c", m=NM, h=8, c=65)
            rv4 = rv_sb[:].rearrange("p (m h c) -> p m h c", m=NM, h=8, c=65)
            nc.vector.memset(v4[:, :, :, 64:65], 1.0)
            nc.vector.memset(rv4[:, :, :, 64:65], 1.0)

            # Score/exp pools opened BEFORE the projection pools so their PSUM
            # banks are disjoint from the projection psum banks.
            spool = ctx.enter_context(tc.tile_pool(name="spool", bufs=2, space="PSUM"))
            ppool = ctx.enter_context(tc.tile_pool(name="ppool", bufs=20))

            p_tiles = {}

            def emit_scores(lqh):
                for dc in range(4):
                    qsl = slice(1024 * dc + 512 * lqh, 1024 * dc + 512 * lqh + 512)
                    for m in range(NM):
                        ksl = slice(LKP * dc + 128 * m, LKP * dc + 128 * m + 128)
                        for br, kt in ((0, kT_sb), (1, rkT_sb)):
                            s = spool.tile([128, 1024], f32, tag="spool", name="s")
                            nc.tensor.matmul(
                                s[:, 0:512], kt[0:64, ksl], qT_sb[0:64, qsl]
                            )
                            nc.tensor.matmul(
                                s[:, 512:1024], kt[64:128, ksl], qT_sb[64:128, qsl]
                            )
                            p = ppool.tile([128, 1024], bf16, tag="ppool", name="p")
                            nc.scalar.activation(
                                p[:],
                                s[:],
                                Exp,
                                bias=maskb_sb[:, m : m + 1],
                                scale=SCALE,
                            )
                            p_tiles[(lqh, dc, m, br)] = p

            # ---------------- Phase 1: projections ----------------
            with ExitStack() as ph1:
                inp = ph1.enter_context(tc.tile_pool(name="inp", bufs=16))
                wch_pool = ph1.enter_context(tc.tile_pool(name="wch", bufs=12))
                ppsum = ph1.enter_context(
                    tc.tile_pool(name="ppsum", bufs=2, space="PSUM")
                )

                # HAM warmup: keep the PE busy while the first input DMAs are
                # in flight so the clock gate is open when real work arrives.
                wp = ppsum.tile([128, 1024], f32, tag="ppsum", name="warmps")
                for i in range(8):
                    nc.tensor.matmul(
                        wp[:, 0:512], warm_sb[:, 0:128], warm_sb[:],
                        skip_group_check=True,
                    )

                # Transposed projections: out chunk dc = lhsT(W block).T @ x_chunk.
                # q runs k-outer over dc pairs so its matmuls start with the
                # first chunk arrivals (keeps the PE trickling + HAM warm).
                for name, xt, wt, b_sb, out_sb, LL, kouter in (
                    ("q", xqT, wqT, bq_sb, qT_sb, LQ, True),
                    ("k", xkT, wkT, bk_sb, kT_sb, LKP, False),
                    ("rk", xrT, wrkT, brk_sb, rkT_sb, LKP, False),
                ):
                    nsl = [slice(a, min(a + 512, LL)) for a in range(0, LL, 512)]
                    xch = []
                    wch = []
                    for k in range(8):
                        # q chunks split across sync/scalar queues so issue
                        # overhead parallelizes and the first chunks land asap.
                        eng = nc.sync if (name != "q" or k < 4) else nc.scalar
                        t = inp.tile([128, LL], bf16, tag="inp", name=f"x{name}{k}")
                        eng.dma_start(
                            out=t[:], in_=xt[128 * k : 128 * k + 128, :]
                        )
                        xch.append(t)
                        w = wch_pool.tile([128, HD], bf16, tag="wch", name=f"w{name}{k}")
                        eng.dma_start(
                            out=w[:], in_=wt[128 * k : 128 * k + 128, :]
                        )
                        wch.append(w)
                    if kouter:
                        for pair in ((0, 1), (2, 3)):
                            pss = {
                                dcq: ppsum.tile(
                                    [128, LL], f32, tag="ppsum", name=f"psq{dcq}"
                                )
                                for dcq in pair
                            }
                            for k in range(8):
                                for dcq in pair:
                                    for sl in nsl:
                                        nc.tensor.matmul(
                                            pss[dcq][:, sl],
                                            wch[k][:, 128 * dcq : 128 * dcq + 128],
                                            xch[k][:, sl],
                                            start=(k == 0),
                                            stop=(k == 7),
                                        )
                            for dcq in pair:
                                nc.vector.tensor_scalar(
                                    out=out_sb[:, LL * dcq : LL * dcq + LL],
                                    in0=pss[dcq][:],
                                    scalar1=b_sb[:, dcq : dcq + 1],
                                    scalar2=None,
                                    op0=Add,
                                )
                        continue
                    for dc in range(4):
                        ps = ppsum.tile([128, LL], f32, tag="ppsum")
                        for k in range(8):
                            for sl in nsl:
                                nc.tensor.matmul(
                                    ps[:, sl],
                                    wch[k][:, 128 * dc : 128 * dc + 128],
                                    xch[k][:, sl],
                                    start=(k == 0),
                                    stop=(k == 7),
                                )
                        nc.vector.tensor_scalar(
                            out=out_sb[:, LL * dc : LL * dc + LL],
                            in0=ps[:],
                            scalar1=b_sb[:, dc : dc + 1],
                            scalar2=None,
                            op0=Add,
                        )

                # Scores/exp for the first lq half can start as soon as the
                # q/k/rk projections land - emit them before v/rv so the ACT
                # engine gets fed during the remaining projections.
                emit_scores(0)

                # Natural-orientation projections for v / rv.
                for name, xt, wt, b_sb, out4 in (
                    ("v", xvT, wvT, bv_sb, v4),
                    ("rv", xrT, wrvT, brv_sb, rv4),
                ):
                    xch = []
                    wch = []
                    for k in range(8):
                        t = inp.tile([128, LKP], bf16, tag="inp", name=f"x{name}{k}")
                        nc.sync.dma_start(
                            out=t[:], in_=xt[128 * k : 128 * k + 128, :]
                        )
                        xch.append(t)
                        w = wch_pool.tile([128, HD], bf16, tag="wch", name=f"w{name}{k}")
                        nc.sync.dma_start(
                            out=w[:], in_=wt[128 * k : 128 * k + 128, :]
                        )
                        wch.append(w)
                    for m in range(NM):
                        ps = ppsum.tile([128, 512], f32, tag="ppsum")
                        for k in range(8):
                            nc.tensor.matmul(
                                ps[:],
                                xch[k][:, 128 * m : 128 * m + 128],
                                wch[k][:],
                                start=(k == 0),
                                stop=(k == 7),
                            )
                        nc.vector.tensor_tensor(
                            out=out4[:, m, :, 0:64],
                            in0=ps[:].rearrange("p (h c) -> p h c", h=8, c=64),
                            in1=b_sb[:].rearrange("p (h c) -> p h c", h=8, c=64),
                            op=Add,
                        )

            emit_scores(1)

            # -------- Phase B: PV accumulation, normalize, output projection ----
            with ExitStack() as ph2:
                xpool = ph2.enter_context(
                    tc.tile_pool(name="xpool", bufs=4, space="PSUM")
                )
                xsb = ph2.enter_context(tc.tile_pool(name="xsb", bufs=8))
                sgp = ph2.enter_context(tc.tile_pool(name="sgp", bufs=2))
                bcp = ph2.enter_context(tc.tile_pool(name="bcp", bufs=4))
                wop = ph2.enter_context(tc.tile_pool(name="wop", bufs=4))
                ysb = ph2.enter_context(tc.tile_pool(name="ysb", bufs=4))

                woch = []
                for dc in range(4):
                    w = wop.tile([128, 1024], bf16, tag="wop", name=f"wo{dc}")
                    nc.scalar.dma_start(
                        out=w[:], in_=woT[128 * dc : 128 * dc + 128, :]
                    )
                    woch.append(w)

                def emit_outproj(lqh, wide=False):
                    if not wide:
                        for ot in range(8):
                            ps = xpool.tile(
                                [128, 512], f32, tag="xpool", name=f"psy{ot}"
                            )
                            for dc in range(4):
                                nc.tensor.matmul(
                                    ps[:],
                                    woch[dc][:, 128 * ot : 128 * ot + 128],
                                    xf_sb[
                                        :,
                                        1024 * dc
                                        + 512 * lqh : 1024 * dc
                                        + 512 * lqh
                                        + 512,
                                    ],
                                    start=(dc == 0),
                                    stop=(dc == 3),
                                )
                            y = ysb.tile([128, 512], f32, tag="ysb")
                            if ot % 2 == 0:
                                nc.vector.tensor_copy(out=y[:], in_=ps[:])
                            else:
                                nc.scalar.copy(out=y[:], in_=ps[:])
                            nc.sync.dma_start(
                                out=yT[
                                    128 * ot : 128 * ot + 128,
                                    512 * lqh : 512 * lqh + 512,
                                ],
                                in_=y[:],
                            )
                        return
                    # Wide variant: all 8 ot accumulators live at once (4 xpool
                    # banks + 2 idle spool slots split in half), dc-outer so only
                    # the final dim-chunk's 8 matmuls wait on the last normalize.
                    pss = []
                    for i in range(4):
                        pss.append(
                            xpool.tile([128, 512], f32, tag="xpool", name=f"psw{i}")
                        )
                    for i in range(2):
                        w2 = spool.tile([128, 1024], f32, tag="spool", name=f"psw2{i}")
                        pss.append(w2[:, 0:512])
                        pss.append(w2[:, 512:1024])
                    for dc in range(4):
                        for ot in range(8):
                            nc.tensor.matmul(
                                pss[ot],
                                woch[dc][:, 128 * ot : 128 * ot + 128],
                                xf_sb[
                                    :,
                                    1024 * dc
                                    + 512 * lqh : 1024 * dc
                                    + 512 * lqh
                                    + 512,
                                ],
                                start=(dc == 0),
                                stop=(dc == 3),
                            )
                    for ot in range(8):
                        y = ysb.tile([128, 512], f32, tag="ysb")
                        if ot % 2 == 0:
                            nc.vector.tensor_copy(out=y[:], in_=pss[ot])
                        else:
                            nc.scalar.copy(out=y[:], in_=pss[ot])
                        nc.sync.dma_start(
                            out=yT[
                                128 * ot : 128 * ot + 128, 512 * lqh : 512 * lqh + 512
                            ],
                            in_=y[:],
                        )

                for lqh in range(2):
                    for dc in range(4):
                        xacc = {}
                        for br in range(2):
                            for hs in range(2):
                                xacc[(br, hs)] = xpool.tile(
                                    [65, 512], f32, tag="xpool", name=f"xacc{br}{hs}"
                                )
                        for m in range(NM):
                            for br, vv in ((0, v4), (1, rv4)):
                                pt = p_tiles[(lqh, dc, m, br)]
                                for hs in range(2):
                                    nc.tensor.matmul(
                                        xacc[(br, hs)][:],
                                        vv[:, m, 2 * dc + hs, :],
                                        pt[:, 512 * hs : 512 * hs + 512],
                                        start=(m == 0),
                                        stop=(m == NM - 1),
                                    )
                        # Copy x accumulators (with sums in row 64) to SBUF,
                        # packed into one tile so the denominator row ships to
                        # DRAM in a single DMA.
                        xs_all = xsb.tile([65, 4 * 512], f32, tag="xsall", bufs=3)
                        xs = {}
                        for j, (br, hs) in enumerate(
                            [(0, 0), (1, 0), (0, 1), (1, 1)]
                        ):
                            sl = xs_all[:, 512 * j : 512 * j + 512]
                            nc.vector.tensor_copy(out=sl, in_=xacc[(br, hs)][:])
                            xs[(br, hs)] = sl
                        # Batch-reciprocate the 4 denominator rows via DRAM.
                        it = 2 * dc + lqh
                        sg = sgp.tile([128, 16], f32, tag="sgp")
                        nc.sync.dma_start(out=scr1[it, :], in_=xs_all[64:65, :])
                        nc.sync.dma_start(out=sg[:], in_=scr1[it, :])
                        nc.vector.reciprocal(sg[:], sg[:])
                        nc.sync.dma_start(out=scr2[it, :], in_=sg[:])
                        for hs in range(2):
                            jv, jr = 2 * hs, 2 * hs + 1
                            bcv = bcp.tile([64, 512], f32, tag="bcp", name="bcv")
                            nc.gpsimd.dma_start(
                                out=bcv[:],
                                in_=scr2[it : it + 1, 512 * jv : 512 * jv + 512]
                                .partition_broadcast(64)[:, 0, :],
                            )
                            bcr = bcp.tile([64, 512], f32, tag="bcp", name="bcr")
                            nc.gpsimd.dma_start(
                                out=bcr[:],
                                in_=scr2[it : it + 1, 512 * jr : 512 * jr + 512]
                                .partition_broadcast(64)[:, 0, :],
                            )
                            t1 = xsb.tile([65, 512], f32, tag="xsb")
                            nc.vector.tensor_tensor(
                                out=t1[0:64, :],
                                in0=xs[(0, hs)][0:64, :],
                                in1=bcv[:],
                                op=Mult,
                            )
                            t2 = xsb.tile([65, 512], f32, tag="xsb")
                            nc.vector.tensor_tensor(
                                out=t2[0:64, :],
                                in0=xs[(1, hs)][0:64, :],
                                in1=bcr[:],
                                op=Mult,
                            )
                            xf_slice = slice(
                                1024 * dc + 512 * lqh, 1024 * dc + 512 * lqh + 512
                            )
                            if hs == 0:
                                nc.vector.tensor_tensor(
                                    out=xf_sb[0:64, xf_slice],
                                    in0=t1[0:64, :],
                                    in1=t2[0:64, :],
                                    op=Add,
                                )
                            else:
                                t3 = xsb.tile([65, 512], bf16, tag="xsb")
                                nc.vector.tensor_tensor(
                                    out=t3[0:64, :],
                                    in0=t1[0:64, :],
                                    in1=t2[0:64, :],
                                    op=Add,
                                )
                                nc.sync.dma_start(
                                    out=xf_sb[64:128, xf_slice],
                                    in_=t3[0:64, :],
                                )
                # outproj(0) emitted AFTER all PV work: its 32 matmuls have no
                # pending dependencies, so they fill the PE stall while the
                # last normalize chains drain; outproj(1) then only waits on
                # the final xf slices.
                emit_outproj(0)
                emit_outproj(1, wide=True)

    nc.compile()
    return nc


def _get_program(lkp=LKP):
    if lkp not in _CACHE:
        _CACHE[lkp] = _build_program(lkp)
    return _CACHE[lkp]


def _bf16(arr):
    import ml_dtypes

    return np.ascontiguousarray(
        np.asarray(arr, dtype=np.float32).astype(ml_dtypes.bfloat16)
    )


def _shard_inputs(inputs, lkp=LKP):
    q = np.ascontiguousarray(inputs["query"], dtype=np.float32)
    k = np.ascontiguousarray(inputs["key"], dtype=np.float32)
    v = np.ascontiguousarray(inputs["value"], dtype=np.float32)
    wr = np.ascontiguousarray(inputs["weak_rela"], dtype=np.float32)
    mask = np.asarray(inputs["mask"])

    in_maps = []
    for c in range(N_CORES):
        b, hh = divmod(c, 2)
        hsl = slice(HD * hh, HD * hh + HD)
        idx = np.nonzero(mask[b, 0])[0]
        nv = len(idx)
        assert nv <= lkp
        pidx = np.concatenate([idx, np.zeros(lkp - nv, dtype=idx.dtype)])
        bias = np.full(lkp, -1.0e9, np.float32)
        bias[:nv] = 0.0
        mb = np.ascontiguousarray(bias.reshape(lkp // 128, 128).T)
        kc, vc, wrc = k[b][pidx], v[b][pidx], wr[b][pidx]
        m = {
            "xqT": _bf16(q[b].T),
            "xkT": _bf16(kc.T),
            "xrT": _bf16(wrc.T),
            "xvT": _bf16(vc.T),
            "wqT": _bf16(np.asarray(inputs["Wq"])[hsl, :].T),
            "wkT": _bf16(np.asarray(inputs["Wk"])[hsl, :].T),
            "wrkT": _bf16(np.asarray(inputs["Wrk"])[hsl, :].T),
            "wvT": _bf16(np.asarray(inputs["Wv"])[hsl, :].T),
            "wrvT": _bf16(np.asarray(inputs["Wrv"])[hsl, :].T),
            "woT": _bf16(np.asarray(inputs["Wo"])[:, hsl].T),
            "bq_pc": np.asarray(inputs["bq"][hsl])
            .reshape(4, 128)
            .T.astype(np.float32),
            "bk_pc": np.asarray(inputs["bk"][hsl])
            .reshape(4, 128)
            .T.astype(np.float32),
            "brk_pc": np.asarray(inputs["brk"][hsl])
            .reshape(4, 128)
            .T.astype(np.float32),
            "bv_bc": np.broadcast_to(inputs["bv"][hsl], (128, HD)).astype(np.float32),
            "brv_bc": np.broadcast_to(inputs["brv"][hsl], (128, HD)).astype(
                np.float32
            ),
            "maskb": mb,
        }
        in_maps.append({k2: np.ascontiguousarray(v2) for k2, v2 in m.items()})
    return in_maps


def run_on_hw(inputs, trace=False, **kw):
    from concourse.bass_utils import run_bass_kernel_spmd

    mask = np.asarray(inputs["mask"])
    max_valid = max(int(mask[b, 0].sum()) for b in range(B))
    lkp = max(LKP, ((max_valid + 127) // 128) * 128)
    nc = _get_program(lkp)
    in_maps = _shard_inputs(inputs, lkp)
    res = run_bass_kernel_spmd(
        nc, in_maps, core_ids=list(range(N_CORES)), trace=trace, **kw
    )
    bo = np.asarray(inputs["bo"], dtype=np.float32)
    outs = []
    for b in range(B):
        yt = res.results[2 * b]["yT"] + res.results[2 * b + 1]["yT"]
        outs.append(yt.T + bo)
    out = np.stack(outs).astype(np.float32)
    return out, res


def kernel(**inputs):
    out, _ = run_on_hw(inputs)
    return out


# revision 35
# speedup vs baseline: 1.0963x; 1.0963x over previous
"""Fused multi-head cross-attention with relation branch, sharded over 8 NeuronCores.

Sharding: data-parallel over batch (4) x tensor-parallel over head halves (2).
Core c handles batch c//2, heads [8*(c%2), 8*(c%2)+8). Each core computes its
partial output projection; the host sums the two partials per batch and adds bo.

Device data flow (per core):
  - q/k/rk projections emitted transposed: qT/kT/rkT [512 local dims, 1024 L]
    (4 chunks of 128 dims = head pairs (2dc, 2dc+1) at partitions 0-63/64-127)
  - v/rv projections emitted natural: [1024 LK, 512 dims], stored per lk-chunk
    with a ones column appended per head ([v_h | 1] of width 65) so the PV
    matmul's row 64 accumulates the softmax denominator for free.
  - scores computed transposed sT[lk, lq] = kT.T @ qT per head, two heads
    row-packed on the PE array (K=64 each at array rows 0-63 / 64-127).
  - exp + mask + 1/sqrt(dk) fused into one ACT op per score tile:
    p = exp(s*scale + bias[lk]) with bias = 0 / -1e9 from the key mask.
  - x_att^T accumulated in PSUM over lk chunks: [v_h|1].T @ p -> [65, lq].
  - softmax denominators batch-reciprocated on 128 DVE lanes via an SBUF->SBUF
    DMA reshape, broadcast over 64 partitions via gpsimd DMAs, then the
    two branches are combined with DVE fma ops.
  - output projection yT = WoT.T @ x_final accumulated over 4 dim chunks.
  - ~8 warmup matmuls on a memset tile right after the preamble keep the PE
    HAM clock gate open while the first input DMAs are still in flight, and
    the input DMA queue leads with xq0/wq0 (split across the sync and scalar
    queues) so real matmuls start as early as possible.
"""

import math

import numpy as np

B, LQ, LK, D, H = 4, 1024, 1024, 1024, 16
DK = D // H
SCALE = 1.0 / math.sqrt(DK)
N_CORES = 8
HD = D // 2  # local dims per core (8 heads * 64)
# Keys are compacted host-side: only unmasked keys are shipped (padded to LKP
# with dummy rows whose mask bias is -1e9, so exp()=0 -> exact same math).
LKP = 640
NM = LKP // 128  # lk chunks

_CACHE = {}


def _build_program(lkp=LKP):
    import concourse.bacc as bacc
    import concourse.mybir as mybir
    import concourse.tile as tile

    LKP = lkp
    NM = LKP // 128

    f32 = mybir.dt.float32
    bf16 = mybir.dt.bfloat16
    Exp = mybir.ActivationFunctionType.Exp
    Add = mybir.AluOpType.add
    Mult = mybir.AluOpType.mult

    nc = bacc.Bacc(
        "TRN2",
        target_bir_lowering=False,
        debug=False,
        enable_asserts=False,
        num_devices=N_CORES,
    )

    # DRAM I/O (per-core shapes; host shards/pre-transposes/casts).
    xqT = nc.dram_tensor("xqT", [D, LQ], bf16, kind="ExternalInput").ap()
    xkT = nc.dram_tensor("xkT", [D, LKP], bf16, kind="ExternalInput").ap()
    xrT = nc.dram_tensor("xrT", [D, LKP], bf16, kind="ExternalInput").ap()
    xvT = nc.dram_tensor("xvT", [D, LKP], bf16, kind="ExternalInput").ap()
    wqT = nc.dram_tensor("wqT", [D, HD], bf16, kind="ExternalInput").ap()
    wkT = nc.dram_tensor("wkT", [D, HD], bf16, kind="ExternalInput").ap()
    wrkT = nc.dram_tensor("wrkT", [D, HD], bf16, kind="ExternalInput").ap()
    wvT = nc.dram_tensor("wvT", [D, HD], bf16, kind="ExternalInput").ap()
    wrvT = nc.dram_tensor("wrvT", [D, HD], bf16, kind="ExternalInput").ap()
    woT = nc.dram_tensor("woT", [HD, D], bf16, kind="ExternalInput").ap()
    bq_pc = nc.dram_tensor("bq_pc", [128, 4], f32, kind="ExternalInput").ap()
    bk_pc = nc.dram_tensor("bk_pc", [128, 4], f32, kind="ExternalInput").ap()
    brk_pc = nc.dram_tensor("brk_pc", [128, 4], f32, kind="ExternalInput").ap()
    bv_bc = nc.dram_tensor("bv_bc", [128, HD], f32, kind="ExternalInput").ap()
    brv_bc = nc.dram_tensor("brv_bc", [128, HD], f32, kind="ExternalInput").ap()
    maskb = nc.dram_tensor("maskb", [128, NM], f32, kind="ExternalInput").ap()
    yT = nc.dram_tensor("yT", [D, LQ], bf16, kind="ExternalOutput").ap()
    scr1 = nc.dram_tensor("scr1", [8, 2048], f32, kind="Internal").ap()
    scr2 = nc.dram_tensor("scr2", [8, 2048], f32, kind="Internal").ap()

    with tile.TileContext(nc) as tc:
        from contextlib import ExitStack

        with ExitStack() as ctx:
            # Persistent SBUF tensors.
            persist = ctx.enter_context(tc.tile_pool(name="persist", bufs=1))
            qT_sb = persist.tile([128, 4 * LQ], bf16, tag="qT")
            kT_sb = persist.tile([128, 4 * LKP], bf16, tag="kT")
            rkT_sb = persist.tile([128, 4 * LKP], bf16, tag="rkT")
            v_sb = persist.tile([128, NM * 8 * 65], bf16, tag="v")
            rv_sb = persist.tile([128, NM * 8 * 65], bf16, tag="rv")
            xf_sb = persist.tile([128, 4 * LQ], bf16, tag="xf")
            maskb_sb = persist.tile([128, NM], f32, tag="maskb")
            bq_sb = persist.tile([128, 4], f32, tag="bq")
            bk_sb = persist.tile([128, 4], f32, tag="bk")
            brk_sb = persist.tile([128, 4], f32, tag="brk")
            bv_sb = persist.tile([128, HD], f32, tag="bv")
            brv_sb = persist.tile([128, HD], f32, tag="brv")
            warm_sb = persist.tile([128, 512], bf16, tag="warm")

            # Memset first: warmup matmuls depend only on this.
            nc.vector.memset(warm_sb[:], 0.125)

            # Small parameter DMAs on the gpsimd queue (off the critical
            # sync queue that carries the first input chunks).
            nc.gpsimd.dma_start(out=maskb_sb[:], in_=maskb)
            nc.gpsimd.dma_start(out=bq_sb[:], in_=bq_pc)
            nc.gpsimd.dma_start(out=bk_sb[:], in_=bk_pc)
            nc.gpsimd.dma_start(out=brk_sb[:], in_=brk_pc)
            nc.gpsimd.dma_start(out=bv_sb[:], in_=bv_bc)
            nc.gpsimd.dma_start(out=brv_sb[:], in_=brv_bc)

            v4 = v_sb[:].rearrange("p (m h c) -> p m h c", m=NM, h=8, c=65)
            rv4 = rv_sb[:].rearrange("p (m h c) -> p m h c", m=NM, h=8, c=65)
            nc.vector.memset(v4[:, :, :, 64:65], 1.0)
            nc.vector.memset(rv4[:, :, :, 64:65], 1.0)

            # Score/exp pools opened BEFORE the projection pools so their PSUM
            # banks are disjoint from the projection psum banks.
            spool = ctx.enter_context(tc.tile_pool(name="spool", bufs=2, space="PSUM"))
            ppool = ctx.enter_context(tc.tile_pool(name="ppool", bufs=20))

            p_tiles = {}

            def emit_scores(lqh):
                for dc in range(4):
                    qsl = slice(1024 * dc + 512 * lqh, 1024 * dc + 512 * lqh + 512)
                    for m in range(NM):
                        ksl = slice(LKP * dc + 128 * m, LKP * dc + 128 * m + 128)
                        for br, kt in ((0, kT_sb), (1, rkT_sb)):
                            s = spool.tile([128, 1024], f32, tag="spool", name="s")
                            nc.tensor.matmul(
                                s[:, 0:512], kt[0:64, ksl], qT_sb[0:64, qsl]
                            )
                            nc.tensor.matmul(
                                s[:, 512:1024], kt[64:128, ksl], qT_sb[64:128, qsl]
                            )
                            p = ppool.tile([128, 1024], bf16, tag="ppool", name="p")
                            nc.scalar.activation(
                                p[:],
                                s[:],
                                Exp,
                                bias=maskb_sb[:, m : m + 1],
                                scale=SCALE,
                            )
                            p_tiles[(lqh, dc, m, br)] = p

            # ---------------- Phase 1: projections ----------------
            with ExitStack() as ph1:
                inp = ph1.enter_context(tc.tile_pool(name="inp", bufs=16))
                wch_pool = ph1.enter_context(tc.tile_pool(name="wch", bufs=12))
                ppsum = ph1.enter_context(
                    tc.tile_pool(name="ppsum", bufs=2, space="PSUM")
                )

                # HAM warmup: keep the PE busy while the first input DMAs are
                # in flight so the clock gate is open when real work arrives.
                wp = ppsum.tile([128, 1024], f32, tag="ppsum", name="warmps")
                for i in range(8):
                    nc.tensor.matmul(
                        wp[:, 0:512], warm_sb[:, 0:128], warm_sb[:],
                        skip_group_check=True,
                    )

                # Transposed projections: out chunk dc = lhsT(W block).T @ x_chunk.
                # q runs k-outer over dc pairs so its matmuls start with the
                # first chunk arrivals (keeps the PE trickling + HAM warm).
                for name, xt, wt, b_sb, out_sb, LL, kouter in (
                    ("q", xqT, wqT, bq_sb, qT_sb, LQ, True),
                    ("k", xkT, wkT, bk_sb, kT_sb, LKP, False),
                    ("rk", xrT, wrkT, brk_sb, rkT_sb, LKP, False),
                ):
                    nsl = [slice(a, min(a + 512, LL)) for a in range(0, LL, 512)]
                    xch = []
                    wch = []
                    for k in range(8):
                        # q chunks split across sync/scalar queues so issue
                        # overhead parallelizes and the first chunks land asap.
                        eng = nc.sync if (name != "q" or k < 4) else nc.scalar
                        t = inp.tile([128, LL], bf16, tag="inp", name=f"x{name}{k}")
                        eng.dma_start(
                            out=t[:], in_=xt[128 * k : 128 * k + 128, :]
                        )
                        xch.append(t)
                        w = wch_pool.tile([128, HD], bf16, tag="wch", name=f"w{name}{k}")
                        eng.dma_start(
                            out=w[:], in_=wt[128 * k : 128 * k + 128, :]
                        )
                        wch.append(w)
                    if kouter:
                        for pair in ((0, 1), (2, 3)):
                            pss = {
                                dcq: ppsum.tile(
                                    [128, LL], f32, tag="ppsum", name=f"psq{dcq}"
                                )
                                for dcq in pair
                            }
                            for k in range(8):
                                for dcq in pair:
                                    for sl in nsl:
                                        nc.tensor.matmul(
                                            pss[dcq][:, sl],
                                            wch[k][:, 128 * dcq : 128 * dcq + 128],
                                            xch[k][:, sl],
                                            start=(k == 0),
                                            stop=(k == 7),
                                        )
                            for dcq in pair:
                                nc.vector.tensor_scalar(
                                    out=out_sb[:, LL * dcq : LL * dcq + LL],
                                    in0=pss[dcq][:],
                                    scalar1=b_sb[:, dcq : dcq + 1],
                                    scalar2=None,
                                    op0=Add,
                                )
                        continue
                    for dc in range(4):
                        ps = ppsum.tile([128, LL], f32, tag="ppsum")
                        for k in range(8):
                            for sl in nsl:
                                nc.tensor.matmul(
                                    ps[:, sl],
                                    wch[k][:, 128 * dc : 128 * dc + 128],
                                    xch[k][:, sl],
                                    start=(k == 0),
                                    stop=(k == 7),
                                )
                        nc.vector.tensor_scalar(
                            out=out_sb[:, LL * dc : LL * dc + LL],
                            in0=ps[:],
                            scalar1=b_sb[:, dc : dc + 1],
                            scalar2=None,
                            op0=Add,
                        )

                # Scores/exp for the first lq half can start as soon as the
                # q/k/rk projections land - emit them before v/rv so the ACT
                # engine gets fed during the remaining projections.
                emit_scores(0)

                # Natural-orientation projections for v / rv.
                for name, xt, wt, b_sb, out4 in (
                    ("v", xvT, wvT, bv_sb, v4),
                    ("rv", xrT, wrvT, brv_sb, rv4),
                ):
                    xch = []
                    wch = []
                    for k in range(8):
                        t = inp.tile([128, LKP], bf16, tag="inp", name=f"x{name}{k}")
                        nc.sync.dma_start(
                            out=t[:], in_=xt[128 * k : 128 * k + 128, :]
                        )
                        xch.append(t)
                        w = wch_pool.tile([128, HD], bf16, tag="wch", name=f"w{name}{k}")
                        nc.sync.dma_start(
                            out=w[:], in_=wt[128 * k : 128 * k + 128, :]
                        )
                        wch.append(w)
                    for m in range(NM):
                        ps = ppsum.tile([128, 512], f32, tag="ppsum")
                        for k in range(8):
                            nc.tensor.matmul(
                                ps[:],
                                xch[k][:, 128 * m : 128 * m + 128],
                                wch[k][:],
                                start=(k == 0),
                                stop=(k == 7),
                            )
                        nc.vector.tensor_tensor(
                            out=out4[:, m, :, 0:64],
                            in0=ps[:].rearrange("p (h c) -> p h c", h=8, c=64),
                            in1=b_sb[:].rearrange("p (h c) -> p h c", h=8, c=64),
                            op=Add,
                        )

            emit_scores(1)

            # -------- Phase B: PV accumulation, normalize, output projection ----
            with ExitStack() as ph2:
                xpool = ph2.enter_context(
                    tc.tile_pool(name="xpool", bufs=4, space="PSUM")
                )
                xsb = ph2.enter_context(tc.tile_pool(name="xsb", bufs=8))
                sgp = ph2.enter_context(tc.tile_pool(name="sgp", bufs=2))
                bcp = ph2.enter_context(tc.tile_pool(name="bcp", bufs=4))
                wop = ph2.enter_context(tc.tile_pool(name="wop", bufs=4))
                ysb = ph2.enter_context(tc.tile_pool(name="ysb", bufs=8))

                woch = []
                for dc in range(4):
                    w = wop.tile([128, 1024], bf16, tag="wop", name=f"wo{dc}")
                    nc.scalar.dma_start(
                        out=w[:], in_=woT[128 * dc : 128 * dc + 128, :]
                    )
                    woch.append(w)

                def emit_outproj(lqh, wide=False):
                    if not wide:
                        for ot in range(8):
                            ps = xpool.tile(
                                [128, 512], f32, tag="xpool", name=f"psy{ot}"
                            )
                            for dc in range(4):
                                nc.tensor.matmul(
                                    ps[:],
                                    woch[dc][:, 128 * ot : 128 * ot + 128],
                                    xf_sb[
                                        :,
                                        1024 * dc
                                        + 512 * lqh : 1024 * dc
                                        + 512 * lqh
                                        + 512,
                                    ],
                                    start=(dc == 0),
                                    stop=(dc == 3),
                                )
                            y = ysb.tile([128, 512], bf16, tag="ysb")
                            if ot % 2 == 0:
                                nc.vector.tensor_copy(out=y[:], in_=ps[:])
                            else:
                                nc.scalar.copy(out=y[:], in_=ps[:])
                            (nc.sync if ot % 2 == 0 else nc.gpsimd).dma_start(
                                out=yT[
                                    128 * ot : 128 * ot + 128,
                                    512 * lqh : 512 * lqh + 512,
                                ],
                                in_=y[:],
                            )
                        return
                    # Wide variant: all 8 ot accumulators live at once (4 xpool
                    # banks + 2 idle spool slots split in half), dc-outer so only
                    # the final dim-chunk's 8 matmuls wait on the last normalize.
                    pss = []
                    for i in range(4):
                        pss.append(
                            xpool.tile([128, 512], f32, tag="xpool", name=f"psw{i}")
                        )
                    for i in range(2):
                        w2 = spool.tile([128, 1024], f32, tag="spool", name=f"psw2{i}")
                        pss.append(w2[:, 0:512])
                        pss.append(w2[:, 512:1024])
                    for dc in range(4):
                        for ot in range(8):
                            nc.tensor.matmul(
                                pss[ot],
                                woch[dc][:, 128 * ot : 128 * ot + 128],
                                xf_sb[
                                    :,
                                    1024 * dc
                                    + 512 * lqh : 1024 * dc
                                    + 512 * lqh
                                    + 512,
                                ],
                                start=(dc == 0),
                                stop=(dc == 3),
                            )
                    for ot in range(8):
                        y = ysb.tile([128, 512], bf16, tag="ysb")
                        if ot % 2 == 0:
                            nc.vector.tensor_copy(out=y[:], in_=pss[ot])
                        else:
                            nc.scalar.copy(out=y[:], in_=pss[ot])
                        (nc.sync if ot % 2 == 0 else nc.gpsimd).dma_start(
                            out=yT[
                                128 * ot : 128 * ot + 128, 512 * lqh : 512 * lqh + 512
                            ],
                            in_=y[:],
                        )

                for lqh in range(2):
                    for dc in range(4):
                        if lqh == 1 and dc == 3:
                            emit_outproj(0)
                        xacc = {}
                        for br in range(2):
                            for hs in range(2):
                                xacc[(br, hs)] = xpool.tile(
                                    [65, 512], f32, tag="xpool", name=f"xacc{br}{hs}"
                                )
                        for m in range(NM):
                            for br, vv in ((0, v4), (1, rv4)):
                                pt = p_tiles[(lqh, dc, m, br)]
                                for hs in range(2):
                                    nc.tensor.matmul(
                                        xacc[(br, hs)][:],
                                        vv[:, m, 2 * dc + hs, :],
                                        pt[:, 512 * hs : 512 * hs + 512],
                                        start=(m == 0),
                                        stop=(m == NM - 1),
                                    )
                        # Copy x accumulators (with sums in row 64) to SBUF,
                        # packed into one tile so the denominator row ships to
                        # DRAM in a single DMA.
                        xs_all = xsb.tile([65, 4 * 512], f32, tag="xsall", bufs=3)
                        xs = {}
                        for j, (br, hs) in enumerate(
                            [(0, 0), (1, 0), (0, 1), (1, 1)]
                        ):
                            sl = xs_all[:, 512 * j : 512 * j + 512]
                            nc.vector.tensor_copy(out=sl, in_=xacc[(br, hs)][:])
                            xs[(br, hs)] = sl
                        # Batch-reciprocate the 4 denominator rows via DRAM.
                        it = 2 * dc + lqh
                        sg = sgp.tile([128, 16], f32, tag="sgp")
                        nc.sync.dma_start(out=scr1[it, :], in_=xs_all[64:65, :])
                        nc.sync.dma_start(out=sg[:], in_=scr1[it, :])
                        nc.vector.reciprocal(sg[:], sg[:])
                        nc.sync.dma_start(out=scr2[it, :], in_=sg[:])
                        for hs in range(2):
                            jv, jr = 2 * hs, 2 * hs + 1
                            bcv = bcp.tile([64, 512], f32, tag="bcp", name="bcv")
                            nc.gpsimd.dma_start(
                                out=bcv[:],
                                in_=scr2[it : it + 1, 512 * jv : 512 * jv + 512]
                                .partition_broadcast(64)[:, 0, :],
                            )
                            bcr = bcp.tile([64, 512], f32, tag="bcp", name="bcr")
                            nc.gpsimd.dma_start(
                                out=bcr[:],
                                in_=scr2[it : it + 1, 512 * jr : 512 * jr + 512]
                                .partition_broadcast(64)[:, 0, :],
                            )
                            t1 = xsb.tile([65, 512], f32, tag="xsb")
                            nc.vector.tensor_tensor(
                                out=t1[0:64, :],
                                in0=xs[(0, hs)][0:64, :],
                                in1=bcv[:],
                                op=Mult,
                            )
                            t2 = xsb.tile([65, 512], f32, tag="xsb")
                            nc.vector.tensor_tensor(
                                out=t2[0:64, :],
                                in0=xs[(1, hs)][0:64, :],
                                in1=bcr[:],
                                op=Mult,
                            )
                            xf_slice = slice(
                                1024 * dc + 512 * lqh, 1024 * dc + 512 * lqh + 512
                            )
                            if hs == 0:
                                nc.vector.tensor_tensor(
                                    out=xf_sb[0:64, xf_slice],
                                    in0=t1[0:64, :],
                                    in1=t2[0:64, :],
                                    op=Add,
                                )
                            else:
                                t3 = xsb.tile([65, 512], bf16, tag="xsb")
                                nc.vector.tensor_tensor(
                                    out=t3[0:64, :],
                                    in0=t1[0:64, :],
                                    in1=t2[0:64, :],
                                    op=Add,
                                )
                                nc.sync.dma_start(
                                    out=xf_sb[64:128, xf_slice],
                                    in_=t3[0:64, :],
                                )
                # outproj(0) emitted AFTER all PV work: its 32 matmuls have no
                # pending dependencies, so they fill the PE stall while the
                # last normalize chains drain; outproj(1) then only waits on
                # the final xf slices.
                emit_outproj(1, wide=True)

    nc.compile()
    return nc


def _get_program(lkp=LKP):
    if lkp not in _CACHE:
        _CACHE[lkp] = _build_program(lkp)
    return _CACHE[lkp]


def _bf16(arr):
    import ml_dtypes

    return np.ascontiguousarray(
        np.asarray(arr, dtype=np.float32).astype(ml_dtypes.bfloat16)
    )


def _shard_inputs(inputs, lkp=LKP):
    q = np.ascontiguousarray(inputs["query"], dtype=np.float32)
    k = np.ascontiguousarray(inputs["key"], dtype=np.float32)
    v = np.ascontiguousarray(inputs["value"], dtype=np.float32)
    wr = np.ascontiguousarray(inputs["weak_rela"], dtype=np.float32)
    mask = np.asarray(inputs["mask"])

    in_maps = []
    for c in range(N_CORES):
        b, hh = divmod(c, 2)
        hsl = slice(HD * hh, HD * hh + HD)
        idx = np.nonzero(mask[b, 0])[0]
        nv = len(idx)
        assert nv <= lkp
        pidx = np.concatenate([idx, np.zeros(lkp - nv, dtype=idx.dtype)])
        bias = np.full(lkp, -1.0e9, np.float32)
        bias[:nv] = 0.0
        mb = np.ascontiguousarray(bias.reshape(lkp // 128, 128).T)
        kc, vc, wrc = k[b][pidx], v[b][pidx], wr[b][pidx]
        m = {
            "xqT": _bf16(q[b].T),
            "xkT": _bf16(kc.T),
            "xrT": _bf16(wrc.T),
            "xvT": _bf16(vc.T),
            "wqT": _bf16(np.asarray(inputs["Wq"])[hsl, :].T),
            "wkT": _bf16(np.asarray(inputs["Wk"])[hsl, :].T),
            "wrkT": _bf16(np.asarray(inputs["Wrk"])[hsl, :].T),
            "wvT": _bf16(np.asarray(inputs["Wv"])[hsl, :].T),
            "wrvT": _bf16(np.asarray(inputs["Wrv"])[hsl, :].T),
            "woT": _bf16(np.asarray(inputs["Wo"])[:, hsl].T),
            "bq_pc": np.asarray(inputs["bq"][hsl])
            .reshape(4, 128)
            .T.astype(np.float32),
            "bk_pc": np.asarray(inputs["bk"][hsl])
            .reshape(4, 128)
            .T.astype(np.float32),
            "brk_pc": np.asarray(inputs["brk"][hsl])
            .reshape(4, 128)
            .T.astype(np.float32),
            "bv_bc": np.broadcast_to(inputs["bv"][hsl], (128, HD)).astype(np.float32),
            "brv_bc": np.broadcast_to(inputs["brv"][hsl], (128, HD)).astype(
                np.float32
            ),
            "maskb": mb,
        }
        in_maps.append({k2: np.ascontiguousarray(v2) for k2, v2 in m.items()})
    return in_maps


def run_on_hw(inputs, trace=False, **kw):
    from concourse.bass_utils import run_bass_kernel_spmd

    mask = np.asarray(inputs["mask"])
    max_valid = max(int(mask[b, 0].sum()) for b in range(B))
    lkp = max(LKP, ((max_valid + 127) // 128) * 128)
    nc = _get_program(lkp)
    in_maps = _shard_inputs(inputs, lkp)
    res = run_bass_kernel_spmd(
        nc, in_maps, core_ids=list(range(N_CORES)), trace=trace, **kw
    )
    bo = np.asarray(inputs["bo"], dtype=np.float32)
    outs = []
    for b in range(B):
        yt = res.results[2 * b]["yT"].astype(np.float32) + res.results[
            2 * b + 1
        ]["yT"].astype(np.float32)
        outs.append(yt.T + bo)
    out = np.stack(outs).astype(np.float32)
    return out, res


def kernel(**inputs):
    out, _ = run_on_hw(inputs)
    return out


# revision 36
# speedup vs baseline: 1.1680x; 1.0654x over previous
"""Fused multi-head cross-attention with relation branch, sharded over 8 NeuronCores.

Sharding: data-parallel over batch (4) x tensor-parallel over head halves (2).
Core c handles batch c//2, heads [8*(c%2), 8*(c%2)+8). Each core computes its
partial output projection; the host sums the two partials per batch and adds bo.

Device data flow (per core):
  - q/k/rk projections emitted transposed: qT/kT/rkT [512 local dims, 1024 L]
    (4 chunks of 128 dims = head pairs (2dc, 2dc+1) at partitions 0-63/64-127)
  - v/rv projections emitted natural: [1024 LK, 512 dims], stored per lk-chunk
    with a ones column appended per head ([v_h | 1] of width 65) so the PV
    matmul's row 64 accumulates the softmax denominator for free.
  - scores computed transposed sT[lk, lq] = kT.T @ qT per head, two heads
    row-packed on the PE array (K=64 each at array rows 0-63 / 64-127).
  - exp + mask + 1/sqrt(dk) fused into one ACT op per score tile:
    p = exp(s*scale + bias[lk]) with bias = 0 / -1e9 from the key mask.
  - x_att^T accumulated in PSUM over lk chunks: [v_h|1].T @ p -> [65, lq].
  - softmax denominators batch-reciprocated on 128 DVE lanes via an SBUF->SBUF
    DMA reshape, broadcast over 64 partitions via gpsimd DMAs, then the
    two branches are combined with DVE fma ops.
  - output projection yT = WoT.T @ x_final accumulated over 4 dim chunks.
  - ~8 warmup matmuls on a memset tile right after the preamble keep the PE
    HAM clock gate open while the first input DMAs are still in flight, and
    the input DMA queue leads with xq0/wq0 (split across the sync and scalar
    queues) so real matmuls start as early as possible.
"""

import math

import numpy as np

B, LQ, LK, D, H = 4, 1024, 1024, 1024, 16
DK = D // H
SCALE = 1.0 / math.sqrt(DK)
N_CORES = 8
HD = D // 2  # local dims per core (8 heads * 64)
# Keys are compacted host-side: only unmasked keys are shipped (padded to LKP
# with dummy rows whose mask bias is -1e9, so exp()=0 -> exact same math).
LKP = 640
NM = LKP // 128  # lk chunks

_CACHE = {}


def _build_program(lkp=LKP):
    import concourse.bacc as bacc
    import concourse.mybir as mybir
    import concourse.tile as tile

    LKP = lkp
    NM = LKP // 128

    f32 = mybir.dt.float32
    bf16 = mybir.dt.bfloat16
    Exp = mybir.ActivationFunctionType.Exp
    Add = mybir.AluOpType.add
    Mult = mybir.AluOpType.mult

    nc = bacc.Bacc(
        "TRN2",
        target_bir_lowering=False,
        debug=False,
        enable_asserts=False,
        num_devices=N_CORES,
    )

    # DRAM I/O (per-core shapes; host shards/pre-transposes/casts).
    xqT = nc.dram_tensor("xqT", [D, LQ], bf16, kind="ExternalInput").ap()
    xkT = nc.dram_tensor("xkT", [D, LKP], bf16, kind="ExternalInput").ap()
    xrT = nc.dram_tensor("xrT", [D, LKP], bf16, kind="ExternalInput").ap()
    xvT = nc.dram_tensor("xvT", [D, LKP], bf16, kind="ExternalInput").ap()
    wqT = nc.dram_tensor("wqT", [D, HD], bf16, kind="ExternalInput").ap()
    wkT = nc.dram_tensor("wkT", [D, HD], bf16, kind="ExternalInput").ap()
    wrkT = nc.dram_tensor("wrkT", [D, HD], bf16, kind="ExternalInput").ap()
    wvT = nc.dram_tensor("wvT", [D, HD], bf16, kind="ExternalInput").ap()
    wrvT = nc.dram_tensor("wrvT", [D, HD], bf16, kind="ExternalInput").ap()
    woT = nc.dram_tensor("woT", [HD, D], bf16, kind="ExternalInput").ap()
    bq_pc = nc.dram_tensor("bq_pc", [128, 4], f32, kind="ExternalInput").ap()
    bk_pc = nc.dram_tensor("bk_pc", [128, 4], f32, kind="ExternalInput").ap()
    brk_pc = nc.dram_tensor("brk_pc", [128, 4], f32, kind="ExternalInput").ap()
    bv_bc = nc.dram_tensor("bv_bc", [128, HD], f32, kind="ExternalInput").ap()
    brv_bc = nc.dram_tensor("brv_bc", [128, HD], f32, kind="ExternalInput").ap()
    maskb = nc.dram_tensor("maskb", [128, NM], f32, kind="ExternalInput").ap()
    yT = nc.dram_tensor("yT", [D, LQ], bf16, kind="ExternalOutput").ap()
    scr1 = nc.dram_tensor("scr1", [8, 2048], f32, kind="Internal").ap()
    scr2 = nc.dram_tensor("scr2", [8, 2048], f32, kind="Internal").ap()

    with tile.TileContext(nc) as tc:
        from contextlib import ExitStack

        with ExitStack() as ctx:
            # Persistent SBUF tensors.
            persist = ctx.enter_context(tc.tile_pool(name="persist", bufs=1))
            qT_sb = persist.tile([128, 4 * LQ], bf16, tag="qT")
            kT_sb = persist.tile([128, 4 * LKP], bf16, tag="kT")
            rkT_sb = persist.tile([128, 4 * LKP], bf16, tag="rkT")
            v_sb = persist.tile([128, NM * 8 * 65], bf16, tag="v")
            rv_sb = persist.tile([128, NM * 8 * 65], bf16, tag="rv")
            xf_sb = persist.tile([128, 4 * LQ], bf16, tag="xf")
            maskb_sb = persist.tile([128, NM], f32, tag="maskb")
            bq_sb = persist.tile([128, 4], f32, tag="bq")
            bk_sb = persist.tile([128, 4], f32, tag="bk")
            brk_sb = persist.tile([128, 4], f32, tag="brk")
            bv_sb = persist.tile([128, HD], f32, tag="bv")
            brv_sb = persist.tile([128, HD], f32, tag="brv")
            warm_sb = persist.tile([128, 512], bf16, tag="warm")

            # Memset first: warmup matmuls depend only on this.
            nc.vector.memset(warm_sb[:], 0.125)

            # Small parameter DMAs on the gpsimd queue (off the critical
            # sync queue that carries the first input chunks).
            nc.gpsimd.dma_start(out=maskb_sb[:], in_=maskb)
            nc.gpsimd.dma_start(out=bq_sb[:], in_=bq_pc)
            nc.gpsimd.dma_start(out=bk_sb[:], in_=bk_pc)
            nc.gpsimd.dma_start(out=brk_sb[:], in_=brk_pc)
            nc.gpsimd.dma_start(out=bv_sb[:], in_=bv_bc)
            nc.gpsimd.dma_start(out=brv_sb[:], in_=brv_bc)

            v4 = v_sb[:].rearrange("p (m h c) -> p m h c", m=NM, h=8, c=65)
            rv4 = rv_sb[:].rearrange("p (m h c) -> p m h c", m=NM, h=8, c=65)
            nc.vector.memset(v4[:, :, :, 64:65], 1.0)
            nc.vector.memset(rv4[:, :, :, 64:65], 1.0)

            # Score/exp pools opened BEFORE the projection pools so their PSUM
            # banks are disjoint from the projection psum banks.
            spool = ctx.enter_context(tc.tile_pool(name="spool", bufs=2, space="PSUM"))
            ppool = ctx.enter_context(tc.tile_pool(name="ppool", bufs=20))

            p_tiles = {}

            def emit_scores(lqh):
                for dc in range(4):
                    qsl = slice(1024 * dc + 512 * lqh, 1024 * dc + 512 * lqh + 512)
                    for m in range(NM):
                        ksl = slice(LKP * dc + 128 * m, LKP * dc + 128 * m + 128)
                        for br, kt in ((0, kT_sb), (1, rkT_sb)):
                            s = spool.tile([128, 1024], f32, tag="spool", name="s")
                            nc.tensor.matmul(
                                s[:, 0:512], kt[0:64, ksl], qT_sb[0:64, qsl]
                            )
                            nc.tensor.matmul(
                                s[:, 512:1024], kt[64:128, ksl], qT_sb[64:128, qsl]
                            )
                            p = ppool.tile([128, 1024], bf16, tag="ppool", name="p")
                            nc.scalar.activation(
                                p[:],
                                s[:],
                                Exp,
                                bias=maskb_sb[:, m : m + 1],
                                scale=SCALE,
                            )
                            p_tiles[(lqh, dc, m, br)] = p

            # ---------------- Phase 1: projections ----------------
            with ExitStack() as ph1:
                inp = ph1.enter_context(tc.tile_pool(name="inp", bufs=16))
                wch_pool = ph1.enter_context(tc.tile_pool(name="wch", bufs=12))
                ppsum = ph1.enter_context(
                    tc.tile_pool(name="ppsum", bufs=2, space="PSUM")
                )

                # HAM warmup: keep the PE busy while the first input DMAs are
                # in flight so the clock gate is open when real work arrives.
                wp = ppsum.tile([128, 1024], f32, tag="ppsum", name="warmps")
                for i in range(8):
                    nc.tensor.matmul(
                        wp[:, 0:512], warm_sb[:, 0:128], warm_sb[:],
                        skip_group_check=True,
                    )

                # Transposed projections: out chunk dc = lhsT(W block).T @ x_chunk.
                # q runs k-outer over dc pairs so its matmuls start with the
                # first chunk arrivals (keeps the PE trickling + HAM warm).
                for name, xt, wt, b_sb, out_sb, LL, kouter in (
                    ("q", xqT, wqT, bq_sb, qT_sb, LQ, True),
                    ("k", xkT, wkT, bk_sb, kT_sb, LKP, False),
                    ("rk", xrT, wrkT, brk_sb, rkT_sb, LKP, False),
                ):
                    nsl = [slice(a, min(a + 512, LL)) for a in range(0, LL, 512)]
                    xch = []
                    wch = []
                    for k in range(8):
                        # q chunks split across sync/scalar queues so issue
                        # overhead parallelizes and the first chunks land asap.
                        eng = nc.sync if (name != "q" or k < 4) else nc.scalar
                        t = inp.tile([128, LL], bf16, tag="inp", name=f"x{name}{k}")
                        eng.dma_start(
                            out=t[:], in_=xt[128 * k : 128 * k + 128, :]
                        )
                        xch.append(t)
                        w = wch_pool.tile([128, HD], bf16, tag="wch", name=f"w{name}{k}")
                        eng.dma_start(
                            out=w[:], in_=wt[128 * k : 128 * k + 128, :]
                        )
                        wch.append(w)
                    if kouter:
                        for pair in ((0, 1), (2, 3)):
                            pss = {
                                dcq: ppsum.tile(
                                    [128, LL], f32, tag="ppsum", name=f"psq{dcq}"
                                )
                                for dcq in pair
                            }
                            for k in range(8):
                                for dcq in pair:
                                    for sl in nsl:
                                        nc.tensor.matmul(
                                            pss[dcq][:, sl],
                                            wch[k][:, 128 * dcq : 128 * dcq + 128],
                                            xch[k][:, sl],
                                            start=(k == 0),
                                            stop=(k == 7),
                                        )
                            for dcq in pair:
                                nc.vector.tensor_scalar(
                                    out=out_sb[:, LL * dcq : LL * dcq + LL],
                                    in0=pss[dcq][:],
                                    scalar1=b_sb[:, dcq : dcq + 1],
                                    scalar2=None,
                                    op0=Add,
                                )
                        continue
                    for dc in range(4):
                        ps = ppsum.tile([128, LL], f32, tag="ppsum")
                        for k in range(8):
                            for sl in nsl:
                                nc.tensor.matmul(
                                    ps[:, sl],
                                    wch[k][:, 128 * dc : 128 * dc + 128],
                                    xch[k][:, sl],
                                    start=(k == 0),
                                    stop=(k == 7),
                                )
                        nc.vector.tensor_scalar(
                            out=out_sb[:, LL * dc : LL * dc + LL],
                            in0=ps[:],
                            scalar1=b_sb[:, dc : dc + 1],
                            scalar2=None,
                            op0=Add,
                        )

                # Scores/exp for the first lq half can start as soon as the
                # q/k/rk projections land - emit them before v/rv so the ACT
                # engine gets fed during the remaining projections.
                emit_scores(0)

                # Natural-orientation projections for v / rv.
                for name, xt, wt, b_sb, out4 in (
                    ("v", xvT, wvT, bv_sb, v4),
                    ("rv", xrT, wrvT, brv_sb, rv4),
                ):
                    xch = []
                    wch = []
                    for k in range(8):
                        t = inp.tile([128, LKP], bf16, tag="inp", name=f"x{name}{k}")
                        nc.sync.dma_start(
                            out=t[:], in_=xt[128 * k : 128 * k + 128, :]
                        )
                        xch.append(t)
                        w = wch_pool.tile([128, HD], bf16, tag="wch", name=f"w{name}{k}")
                        nc.sync.dma_start(
                            out=w[:], in_=wt[128 * k : 128 * k + 128, :]
                        )
                        wch.append(w)
                    for m in range(NM):
                        ps = ppsum.tile([128, 512], f32, tag="ppsum")
                        for k in range(8):
                            nc.tensor.matmul(
                                ps[:],
                                xch[k][:, 128 * m : 128 * m + 128],
                                wch[k][:],
                                start=(k == 0),
                                stop=(k == 7),
                            )
                        nc.vector.tensor_tensor(
                            out=out4[:, m, :, 0:64],
                            in0=ps[:].rearrange("p (h c) -> p h c", h=8, c=64),
                            in1=b_sb[:].rearrange("p (h c) -> p h c", h=8, c=64),
                            op=Add,
                        )

            emit_scores(1)

            # -------- Phase B: PV accumulation, normalize, output projection ----
            with ExitStack() as ph2:
                xpool = ph2.enter_context(
                    tc.tile_pool(name="xpool", bufs=4, space="PSUM")
                )
                xsb = ph2.enter_context(tc.tile_pool(name="xsb", bufs=8))
                sgp = ph2.enter_context(tc.tile_pool(name="sgp", bufs=2))
                bcp = ph2.enter_context(tc.tile_pool(name="bcp", bufs=4))
                wop = ph2.enter_context(tc.tile_pool(name="wop", bufs=4))
                ysb = ph2.enter_context(tc.tile_pool(name="ysb", bufs=8))

                woch = []
                for dc in range(4):
                    w = wop.tile([128, 1024], bf16, tag="wop", name=f"wo{dc}")
                    nc.scalar.dma_start(
                        out=w[:], in_=woT[128 * dc : 128 * dc + 128, :]
                    )
                    woch.append(w)

                def emit_outproj(lqh, wide=False):
                    if not wide:
                        for ot in range(8):
                            ps = xpool.tile(
                                [128, 512], f32, tag="xpool", name=f"psy{ot}"
                            )
                            for dc in range(4):
                                nc.tensor.matmul(
                                    ps[:],
                                    woch[dc][:, 128 * ot : 128 * ot + 128],
                                    xf_sb[
                                        :,
                                        1024 * dc
                                        + 512 * lqh : 1024 * dc
                                        + 512 * lqh
                                        + 512,
                                    ],
                                    start=(dc == 0),
                                    stop=(dc == 3),
                                )
                            y = ysb.tile([128, 512], bf16, tag="ysb")
                            if ot % 2 == 0:
                                nc.vector.tensor_copy(out=y[:], in_=ps[:])
                            else:
                                nc.scalar.copy(out=y[:], in_=ps[:])
                            (nc.sync if ot % 2 == 0 else nc.gpsimd).dma_start(
                                out=yT[
                                    128 * ot : 128 * ot + 128,
                                    512 * lqh : 512 * lqh + 512,
                                ],
                                in_=y[:],
                            )
                        return
                    # Wide variant: all 8 ot accumulators live at once (4 xpool
                    # banks + 2 idle spool slots split in half), dc-outer so only
                    # the final dim-chunk's 8 matmuls wait on the last normalize.
                    pss = []
                    for i in range(4):
                        pss.append(
                            xpool.tile([128, 512], f32, tag="xpool", name=f"psw{i}")
                        )
                    for i in range(2):
                        w2 = spool.tile([128, 1024], f32, tag="spool", name=f"psw2{i}")
                        pss.append(w2[:, 0:512])
                        pss.append(w2[:, 512:1024])
                    for dc in range(4):
                        for ot in range(8):
                            nc.tensor.matmul(
                                pss[ot],
                                woch[dc][:, 128 * ot : 128 * ot + 128],
                                xf_sb[
                                    :,
                                    1024 * dc
                                    + 512 * lqh : 1024 * dc
                                    + 512 * lqh
                                    + 512,
                                ],
                                start=(dc == 0),
                                stop=(dc == 3),
                            )
                    for ot in range(8):
                        y = ysb.tile([128, 512], bf16, tag="ysb")
                        if ot % 2 == 0:
                            nc.vector.tensor_copy(out=y[:], in_=pss[ot])
                        else:
                            nc.scalar.copy(out=y[:], in_=pss[ot])
                        (nc.sync if ot % 2 == 0 else nc.gpsimd).dma_start(
                            out=yT[
                                128 * ot : 128 * ot + 128, 512 * lqh : 512 * lqh + 512
                            ],
                            in_=y[:],
                        )

                for lqh in range(2):
                    for dc in range(4):
                        xacc = {}
                        for br in range(2):
                            for hs in range(2):
                                xacc[(br, hs)] = xpool.tile(
                                    [65, 512], f32, tag="xpool", name=f"xacc{br}{hs}"
                                )
                        for m in range(NM):
                            for br, vv in ((0, v4), (1, rv4)):
                                pt = p_tiles[(lqh, dc, m, br)]
                                for hs in range(2):
                                    nc.tensor.matmul(
                                        xacc[(br, hs)][:],
                                        vv[:, m, 2 * dc + hs, :],
                                        pt[:, 512 * hs : 512 * hs + 512],
                                        start=(m == 0),
                                        stop=(m == NM - 1),
                                    )
                        # Copy x accumulators (with sums in row 64) to SBUF,
                        # packed into one tile so the denominator row ships to
                        # DRAM in a single DMA.
                        xs_all = xsb.tile([65, 4 * 512], f32, tag="xsall", bufs=3)
                        xs = {}
                        for j, (br, hs) in enumerate(
                            [(0, 0), (1, 0), (0, 1), (1, 1)]
                        ):
                            sl = xs_all[:, 512 * j : 512 * j + 512]
                            nc.vector.tensor_copy(out=sl, in_=xacc[(br, hs)][:])
                            xs[(br, hs)] = sl
                        # Batch-reciprocate the 4 denominator rows via DRAM.
                        it = 2 * dc + lqh
                        sg = sgp.tile([128, 16], f32, tag="sgp")
                        nc.sync.dma_start(out=scr1[it, :], in_=xs_all[64:65, :])
                        nc.sync.dma_start(out=sg[:], in_=scr1[it, :])
                        nc.vector.reciprocal(sg[:], sg[:])
                        nc.sync.dma_start(out=scr2[it, :], in_=sg[:])
                        for hs in range(2):
                            jv, jr = 2 * hs, 2 * hs + 1
                            bcv = bcp.tile([64, 512], f32, tag="bcp", name="bcv")
                            nc.gpsimd.dma_start(
                                out=bcv[:],
                                in_=scr2[it : it + 1, 512 * jv : 512 * jv + 512]
                                .partition_broadcast(64)[:, 0, :],
                            )
                            bcr = bcp.tile([64, 512], f32, tag="bcp", name="bcr")
                            nc.gpsimd.dma_start(
                                out=bcr[:],
                                in_=scr2[it : it + 1, 512 * jr : 512 * jr + 512]
                                .partition_broadcast(64)[:, 0, :],
                            )
                            t1 = xsb.tile([65, 512], f32, tag="xsb")
                            nc.vector.tensor_tensor(
                                out=t1[0:64, :],
                                in0=xs[(0, hs)][0:64, :],
                                in1=bcv[:],
                                op=Mult,
                            )
                            t2 = xsb.tile([65, 512], f32, tag="xsb")
                            nc.vector.tensor_tensor(
                                out=t2[0:64, :],
                                in0=xs[(1, hs)][0:64, :],
                                in1=bcr[:],
                                op=Mult,
                            )
                            xf_slice = slice(
                                1024 * dc + 512 * lqh, 1024 * dc + 512 * lqh + 512
                            )
                            if hs == 0:
                                nc.vector.tensor_tensor(
                                    out=xf_sb[0:64, xf_slice],
                                    in0=t1[0:64, :],
                                    in1=t2[0:64, :],
                                    op=Add,
                                )
                            else:
                                t3 = xsb.tile([65, 512], bf16, tag="xsb")
                                nc.vector.tensor_tensor(
                                    out=t3[0:64, :],
                                    in0=t1[0:64, :],
                                    in1=t2[0:64, :],
                                    op=Add,
                                )
                                nc.sync.dma_start(
                                    out=xf_sb[64:128, xf_slice],
                                    in_=t3[0:64, :],
                                )
                # outproj(0) emitted AFTER all PV work: its 32 matmuls have no
                # pending dependencies, so they fill the PE stall while the
                # last normalize chains drain; outproj(1) then only waits on
                # the final xf slices.
                emit_outproj(0)
                emit_outproj(1, wide=True)

    nc.compile()
    return nc


def _get_program(lkp=LKP):
    if lkp not in _CACHE:
        _CACHE[lkp] = _build_program(lkp)
    return _CACHE[lkp]


def _bf16(arr):
    import ml_dtypes

    return np.ascontiguousarray(
        np.asarray(arr, dtype=np.float32).astype(ml_dtypes.bfloat16)
    )


def _shard_inputs(inputs, lkp=LKP):
    q = np.ascontiguousarray(inputs["query"], dtype=np.float32)
    k = np.ascontiguousarray(inputs["key"], dtype=np.float32)
    v = np.ascontiguousarray(inputs["value"], dtype=np.float32)
    wr = np.ascontiguousarray(inputs["weak_rela"], dtype=np.float32)
    mask = np.asarray(inputs["mask"])

    in_maps = []
    for c in range(N_CORES):
        b, hh = divmod(c, 2)
        hsl = slice(HD * hh, HD * hh + HD)
        idx = np.nonzero(mask[b, 0])[0]
        nv = len(idx)
        assert nv <= lkp
        pidx = np.concatenate([idx, np.zeros(lkp - nv, dtype=idx.dtype)])
        bias = np.full(lkp, -1.0e9, np.float32)
        bias[:nv] = 0.0
        mb = np.ascontiguousarray(bias.reshape(lkp // 128, 128).T)
        kc, vc, wrc = k[b][pidx], v[b][pidx], wr[b][pidx]
        m = {
            "xqT": _bf16(q[b].T),
            "xkT": _bf16(kc.T),
            "xrT": _bf16(wrc.T),
            "xvT": _bf16(vc.T),
            "wqT": _bf16(np.asarray(inputs["Wq"])[hsl, :].T),
            "wkT": _bf16(np.asarray(inputs["Wk"])[hsl, :].T),
            "wrkT": _bf16(np.asarray(inputs["Wrk"])[hsl, :].T),
            "wvT": _bf16(np.asarray(inputs["Wv"])[hsl, :].T),
            "wrvT": _bf16(np.asarray(inputs["Wrv"])[hsl, :].T),
            "woT": _bf16(np.asarray(inputs["Wo"])[:, hsl].T),
            "bq_pc": np.asarray(inputs["bq"][hsl])
            .reshape(4, 128)
            .T.astype(np.float32),
            "bk_pc": np.asarray(inputs["bk"][hsl])
            .reshape(4, 128)
            .T.astype(np.float32),
            "brk_pc": np.asarray(inputs["brk"][hsl])
            .reshape(4, 128)
            .T.astype(np.float32),
            "bv_bc": np.broadcast_to(inputs["bv"][hsl], (128, HD)).astype(np.float32),
            "brv_bc": np.broadcast_to(inputs["brv"][hsl], (128, HD)).astype(
                np.float32
            ),
            "maskb": mb,
        }
        in_maps.append({k2: np.ascontiguousarray(v2) for k2, v2 in m.items()})
    return in_maps


def run_on_hw(inputs, trace=False, **kw):
    from concourse.bass_utils import run_bass_kernel_spmd

    mask = np.asarray(inputs["mask"])
    max_valid = max(int(mask[b, 0].sum()) for b in range(B))
    lkp = max(LKP, ((max_valid + 127) // 128) * 128)
    nc = _get_program(lkp)
    in_maps = _shard_inputs(inputs, lkp)
    res = run_bass_kernel_spmd(
        nc, in_maps, core_ids=list(range(N_CORES)), trace=trace, **kw
    )
    bo = np.asarray(inputs["bo"], dtype=np.float32)
    outs = []
    for b in range(B):
        yt = res.results[2 * b]["yT"].astype(np.float32) + res.results[
            2 * b + 1
        ]["yT"].astype(np.float32)
        outs.append(yt.T + bo)
    out = np.stack(outs).astype(np.float32)
    return out, res


def kernel(**inputs):
    out, _ = run_on_hw(inputs)
    return out
